# revision 6
# baseline (speedup 1.0000x reference)
"""Compositional attention Trainium2 Bass kernel (V5).

Sharding: 8 cores = 2 batches x 4 search-pairs.
Core c handles batch b=c//4 and searches (2*(c%4), 2*(c%4)+1); each core
produces a partial output for its 128 columns of the S*D=512 concat dim
(host sums 4 partials per batch).

V5 notes (over V4):
  - The two searches are staggered by one query block: slot t runs
    s0 on block t and s1 on block t-1. s0 finishes a slot early, so its
    whole per-query scalar chain (row reduce -> bounce -> sigmoid ->
    broadcast -> blend) overlaps s1's last attention block.
  - All four per-query row reduces of a slot (sums/diff x both
    searches) land in ONE PSUM bank on partitions 0/32/64/96 via
    distinct PE col groups, freeing banks so the chain runs inside the
    attention loop (8-bank budget: scores 4 + retrieve 2 + dW 1 + rows 1).
  - Weight DMAs are spread across the scalar/gpsimd queues so the first
    projection matmul starts right after x chunk 0 lands.
  - Output projection uses 6 PSUM banks of run-ahead.
"""

import sys

for _p in ("/opt/trn_rl_repo",):
    if _p not in sys.path:
        sys.path.insert(0, _p)

from contextlib import ExitStack

import ml_dtypes
import numpy as np

import concourse.bass as bass
import concourse.tile as tile
from concourse import bacc
from concourse import mybir
from concourse.bass import ts
from concourse.bass_utils import run_bass_kernel_spmd

B, N, DIM, S, R, D = 2, 2048, 1024, 8, 2, 64
NCORES = 8
SPC = 2          # searches per core
SD = SPC * D     # 128 (per-core slice of S*D)
RD = R * D       # 128
P = 128
IBL = 512        # query block
NIB = N // IBL   # 4
KC = DIM // P    # 8
NJT = N // P     # 16 key tiles
F32 = mybir.dt.float32
BF16 = mybir.dt.bfloat16
SCALE = float(D) ** -0.5
AF = mybir.ActivationFunctionType
ALU = mybir.AluOpType


def _emit(ctx: ExitStack, tc: tile.TileContext, io):
    nc = tc.nc
    xT, wq, wk, wr, wv, wrk, wout, outp = io

    singles = ctx.enter_context(tc.tile_pool(name="singles", bufs=1))
    ones_b = singles.tile([P, 1], BF16)
    nc.vector.memset(ones_b, 1.0)

    wq_sb = singles.tile([P, KC, SD], BF16)
    wk_sb = singles.tile([P, KC, SD], BF16)
    wr_sb = singles.tile([P, KC, SD], BF16)
    wv_sb = singles.tile([P, KC, RD], BF16)
    wrk2 = singles.tile([P, D], BF16)   # Wrk twice: rows 0:64 and 64:128
    wout_sb = singles.tile([P, DIM], BF16)

    acts = ctx.enter_context(tc.tile_pool(name="acts", bufs=1))
    qT = acts.tile([P, N], BF16)
    kT = acts.tile([P, N], BF16)
    rqT = acts.tile([P, N], BF16)
    vnat = acts.tile([P, NJT, RD], BF16)   # [key-part, key-tile, r*d]
    ret0 = acts.tile([P, N], BF16)         # search0 retrievedT (unnormalized)
    ret1 = acts.tile([P, N], BF16)         # search1
    rsh = acts.tile([P, N], BF16)          # [0:64]=s0 r1 shifted dn, [64:128]=s1 r0 up
    dT = acts.tile([P, N], BF16)           # r0-r1 per search (s0 rows 0:64)
    dprod = acts.tile([P, N], BF16)        # rq * (d @ Wrk)
    red0 = acts.tile([P, N], BF16)         # per-key-part exp sums
    red1 = acts.tile([P, N], BF16)
    bc0 = acts.tile([P, N], BF16)          # broadcast sig*inv
    bc1 = acts.tile([P, N], BF16)          # broadcast inv
    comp = acts.tile([P, N], BF16)
    rets = (ret0, ret1)
    reds = (red0, red1)

    # ---------------- projections ----------------
    with tc.tile_pool(name="xpool", bufs=1) as xpool, \
         tc.tile_pool(name="pja", bufs=1, space="PSUM") as pja, \
         tc.tile_pool(name="pjb", bufs=2, space="PSUM") as pjb:
        xs = xpool.tile([P, KC, N], BF16)
        xr = xT.rearrange("(kc p) n -> p kc n", p=P)
        # x chunk 0 + pass-A weights first, each on its own queue
        nc.sync.dma_start(out=xs[:, 0, :], in_=xr[:, 0, :])
        nc.scalar.dma_start(out=wk_sb,
                            in_=wk.rearrange("(kc p) m -> p kc m", p=P))
        nc.gpsimd.dma_start(out=wv_sb,
                            in_=wv.rearrange("(kc p) m -> p kc m", p=P))
        for k in range(1, KC):
            nc.sync.dma_start(out=xs[:, k, :], in_=xr[:, k, :])
        nc.scalar.dma_start(out=wq_sb,
                            in_=wq.rearrange("(kc p) m -> p kc m", p=P))
        nc.gpsimd.dma_start(out=wr_sb,
                            in_=wr.rearrange("(kc p) m -> p kc m", p=P))
        nc.scalar.dma_start(out=wrk2[0:64, :], in_=wrk)
        nc.scalar.dma_start(out=wrk2[64:128, :], in_=wrk)
        nc.gpsimd.dma_start(out=wout_sb, in_=wout)

        vtmp = xpool.tile([P, N], BF16)
        # pass A: kT + vT (8 banks), k-ordered so MM k waits only chunk k
        kps = [pja.tile([P, IBL], F32, tag="pk", name=f"pk{ib}")
               for ib in range(NIB)]
        vps = [pja.tile([P, IBL], F32, tag="pv", name=f"pv{ib}")
               for ib in range(NIB)]
        for k in range(KC):
            for ib in range(NIB):
                nc.tensor.matmul(kps[ib], lhsT=wk_sb[:, k, :],
                                 rhs=xs[:, k, ts(ib, IBL)],
                                 start=(k == 0), stop=(k == KC - 1))
                nc.tensor.matmul(vps[ib], lhsT=wv_sb[:, k, :],
                                 rhs=xs[:, k, ts(ib, IBL)],
                                 start=(k == 0), stop=(k == KC - 1))
        for ib in range(NIB):
            nc.vector.tensor_copy(out=kT[:, ts(ib, IBL)], in_=kps[ib])
            nc.scalar.copy(out=vtmp[:, ts(ib, IBL)], in_=vps[ib])
            for h in range(IBL // P):
                jt = ib * (IBL // P) + h
                nc.scalar.dma_start_transpose(vnat[:, jt, :], vtmp[:, ts(jt, P)])
        # pass B: qT + rqT, ib-ordered so attention slot 0 can start early
        for ib in range(NIB):
            qp = pjb.tile([P, IBL], F32, tag="pq", name="pq")
            rp = pjb.tile([P, IBL], F32, tag="pr", name="pr")
            for k in range(KC):
                nc.tensor.matmul(qp, lhsT=wq_sb[:, k, :],
                                 rhs=xs[:, k, ts(ib, IBL)],
                                 start=(k == 0), stop=(k == KC - 1))
                nc.tensor.matmul(rp, lhsT=wr_sb[:, k, :],
                                 rhs=xs[:, k, ts(ib, IBL)],
                                 start=(k == 0), stop=(k == KC - 1))
            nc.vector.tensor_copy(out=qT[:, ts(ib, IBL)], in_=qp)
            nc.scalar.copy(out=rqT[:, ts(ib, IBL)], in_=rp)

    # DRAM bounce buffers for per-query scalars
    dramp = ctx.enter_context(tc.tile_pool(name="dramp", bufs=1, space="DRAM"))
    sums_dr = [dramp.tile([N], F32, tag=f"sums{si}", name=f"sums{si}")
               for si in range(SPC)]
    diff_dr = [dramp.tile([N], F32, tag=f"diff{si}", name=f"diff{si}")
               for si in range(SPC)]
    a0_dr = [dramp.tile([N], BF16, tag=f"a0{si}", name=f"a0d{si}")
             for si in range(SPC)]
    a1_dr = [dramp.tile([N], BF16, tag=f"a1{si}", name=f"a1d{si}")
             for si in range(SPC)]

    etmp = ctx.enter_context(tc.tile_pool(name="etmp", bufs=2))
    s128 = [etmp.tile([P, N // P], F32, tag=f"s128_{si}", name=f"s128_{si}")
            for si in range(SPC)]
    d128 = [etmp.tile([P, N // P], F32, tag=f"d128_{si}", name=f"d128_{si}")
            for si in range(SPC)]
    inv = [etmp.tile([P, N // P], F32, tag=f"inv{si}", name=f"inv{si}")
           for si in range(SPC)]
    t16 = [etmp.tile([P, N // P], F32, tag=f"t16_{si}", name=f"t16_{si}")
           for si in range(SPC)]
    ra0 = [etmp.tile([P, N // P], F32, tag=f"ra0_{si}", name=f"ra0_{si}")
           for si in range(SPC)]
    a0b = [etmp.tile([P, N // P], BF16, tag=f"a0b{si}", name=f"a0b{si}")
           for si in range(SPC)]
    a1b = [etmp.tile([P, N // P], BF16, tag=f"a1b{si}", name=f"a1b{si}")
           for si in range(SPC)]
    t1 = etmp.tile([P, N], BF16, tag="t1")
    t2 = etmp.tile([P, N], BF16, tag="t2")

    def chain(si, dq):
        """Per-query scalar chain + blend for one search. Emitted where it
        should overlap other work; dq is the DMA queue to ride."""
        lo = 64 * si
        dq.dma_start(out=s128[si],
                     in_=sums_dr[si].rearrange("(p f) -> p f", p=P))
        dq.dma_start(out=d128[si],
                     in_=diff_dr[si].rearrange("(p f) -> p f", p=P))
        nc.vector.reciprocal(inv[si], s128[si])
        nc.vector.tensor_tensor(t16[si], d128[si], inv[si], ALU.mult)
        nc.scalar.activation(out=ra0[si], in_=t16[si], func=AF.Sigmoid,
                             scale=SCALE)
        nc.vector.tensor_tensor(a0b[si], ra0[si], inv[si], ALU.mult)
        nc.vector.tensor_copy(out=a1b[si], in_=inv[si])
        dq.dma_start(out=a0_dr[si].rearrange("(p f) -> p f", p=P),
                     in_=a0b[si])
        dq.dma_start(out=a1_dr[si].rearrange("(p f) -> p f", p=P),
                     in_=a1b[si])
        dq.dma_start(out=bc0[lo:lo + 64, :],
                     in_=a0_dr[si][None, :].to_broadcast([64, N]))
        nc.gpsimd.dma_start(out=bc1[lo:lo + 64, :],
                            in_=a1_dr[si][None, :].to_broadcast([64, N]))
        # comp = inv*r1 + (sig*inv)*(r0-r1)
        r1ap = rsh[0:64, :] if si == 0 else ret1[64:128, :]
        nc.vector.tensor_tensor(t1[lo:lo + 64, :], bc0[lo:lo + 64, :],
                                dT[lo:lo + 64, :], ALU.mult)
        nc.gpsimd.tensor_tensor(t2[lo:lo + 64, :], bc1[lo:lo + 64, :],
                                r1ap, ALU.mult)
        nc.vector.tensor_tensor(comp[lo:lo + 64, :], t1[lo:lo + 64, :],
                                t2[lo:lo + 64, :], ALU.add)

    # ---------------- attention (staggered searches) ----------------
    # slot t: s0 on block t, s1 on block t-1. PSUM: scores 4 + ret 2
    # + dW 1 + rows 1 = 8 banks.
    with tc.tile_pool(name="expp", bufs=2) as expp, \
         tc.tile_pool(name="trp1", bufs=2) as trp1, \
         tc.tile_pool(name="trp2", bufs=2) as trp2, \
         tc.tile_pool(name="trp3", bufs=2) as trp3, \
         tc.tile_pool(name="scp", bufs=2, space="PSUM") as scp, \
         tc.tile_pool(name="retp", bufs=1, space="PSUM") as retp, \
         tc.tile_pool(name="dwp", bufs=1, space="PSUM") as dwp, \
         tc.tile_pool(name="rowp", bufs=1, space="PSUM") as rowp:
        for t in range(NIB + 1):
            act = []            # [(si, ib)] active this slot
            if t < NIB:
                act.append((0, t))
            if t > 0:
                act.append((1, t - 1))
            ets = expp.tile([P, NJT, SPC, IBL], BF16, tag="exp", name="exp")
            rt = {si: retp.tile([P, IBL], F32, tag=f"rt{si}", name=f"rt{si}")
                  for si, _ in act}
            for jt in range(NJT):
                sp = scp.tile([P, SPC, IBL], F32, tag="sc", name="sc")
                for si, ib in act:
                    lo = 64 * si
                    nc.tensor.matmul(
                        sp[:, si, :],
                        lhsT=kT[lo:lo + 64, ts(jt, P)],
                        rhs=qT[lo:lo + 64, ts(ib, IBL)],
                        start=True, stop=True,
                    )
                if len(act) == 2:
                    nc.scalar.activation(out=ets[:, jt, :, :], in_=sp,
                                         func=AF.Exp, scale=SCALE)
                else:
                    si = act[0][0]
                    nc.scalar.activation(out=ets[:, jt, si:si + 1, :],
                                         in_=sp[:, si:si + 1, :],
                                         func=AF.Exp, scale=SCALE)
                for si, ib in act:
                    nc.tensor.matmul(
                        rt[si], lhsT=vnat[:, jt, :], rhs=ets[:, jt, si, :],
                        start=(jt == 0), stop=(jt == NJT - 1),
                        skip_group_check=True,
                    )
            dwps = dwp.tile([P, IBL], F32, tag="dw", name="dw")
            rowsps = rowp.tile([P, IBL], F32, tag="rows", name="rowsps")
            rowsb = etmp.tile([P, IBL], F32, tag="rowsb", name="rowsb")
            for si, ib in act:
                lo = 64 * si
                ibs = ts(ib, IBL)
                # retrieved evac + partition shift + d = r0 - r1
                if si == 0:
                    nc.vector.tensor_copy(out=ret0[:, ibs], in_=rt[0])
                    nc.gpsimd.dma_start(out=rsh[0:64, ibs],
                                        in_=ret0[64:128, ibs])
                    nc.vector.tensor_tensor(dT[0:64, ibs], ret0[0:64, ibs],
                                            rsh[0:64, ibs], ALU.subtract)
                else:
                    nc.scalar.copy(out=ret1[:, ibs], in_=rt[1])
                    nc.gpsimd.dma_start(out=rsh[64:128, ibs],
                                        in_=ret1[0:64, ibs])
                    nc.vector.tensor_tensor(dT[64:128, ibs],
                                            rsh[64:128, ibs],
                                            ret1[64:128, ibs], ALU.subtract)
                # dW = Wrk^T @ d (diagonal tiles), dprod = rq * dW
                nc.tensor.matmul(dwps[lo:lo + 64, :], lhsT=wrk2[lo:lo + 64, :],
                                 rhs=dT[lo:lo + 64, ibs], start=True, stop=True)
                nc.vector.tensor_tensor(dprod[lo:lo + 64, ibs],
                                        rqT[lo:lo + 64, ibs],
                                        dwps[lo:lo + 64, :], ALU.mult)
                # denominator tree: lvl1+2 DVE, lvl3+4 GpSimd
                g1 = trp1.tile([P, NJT // 2, IBL], BF16, tag="g1",
                               name=f"g1_{si}")
                nc.vector.tensor_tensor(g1, ets[:, 0:8, si, :],
                                        ets[:, 8:16, si, :], ALU.add)
                g2 = trp2.tile([P, NJT // 4, IBL], BF16, tag="g2",
                               name=f"g2_{si}")
                nc.vector.tensor_tensor(g2, g1[:, 0:4, :], g1[:, 4:8, :],
                                        ALU.add)
                g3 = trp3.tile([P, NJT // 8, IBL], BF16, tag="g3",
                               name=f"g3_{si}")
                nc.gpsimd.tensor_tensor(g3, g2[:, 0:2, :], g2[:, 2:4, :],
                                        ALU.add)
                nc.gpsimd.tensor_tensor(reds[si][:, ibs],
                                        g3[:, 0, :], g3[:, 1, :], ALU.add)
                # per-query rows into one bank: sums@(col 64*si),
                # diff@(col 64*si+32)
                nc.tensor.matmul(rowsps[lo:lo + 1, :], lhsT=ones_b,
                                 rhs=reds[si][:, ibs], start=True, stop=True,
                                 tile_position=(0, lo))
                nc.tensor.matmul(rowsps[lo + 32:lo + 33, :],
                                 lhsT=ones_b[lo:lo + 64, :],
                                 rhs=dprod[lo:lo + 64, ibs], start=True,
                                 stop=True, tile_position=(lo, lo + 32))
                if si == 0:
                    nc.vector.tensor_copy(out=rowsb[0:1, :],
                                          in_=rowsps[0:1, :])
                    nc.vector.tensor_copy(out=rowsb[32:33, :],
                                          in_=rowsps[32:33, :])
                    nc.sync.dma_start(out=sums_dr[0][None, ibs],
                                      in_=rowsb[0:1, :])
                    nc.sync.dma_start(out=diff_dr[0][None, ibs],
                                      in_=rowsb[32:33, :])
                else:
                    nc.scalar.copy(out=rowsb[64:65, :], in_=rowsps[64:65, :])
                    nc.scalar.copy(out=rowsb[96:97, :], in_=rowsps[96:97, :])
                    nc.scalar.dma_start(out=sums_dr[1][None, ibs],
                                        in_=rowsb[64:65, :])
                    nc.scalar.dma_start(out=diff_dr[1][None, ibs],
                                        in_=rowsb[96:97, :])
            if t == NIB - 1:
                # s0's rows all written: its chain overlaps the last slot
                chain(0, nc.sync)
        chain(1, nc.scalar)

    # ---------------- output projection ----------------
    ops = ctx.enter_context(tc.tile_pool(name="ops", bufs=3, space="PSUM"))
    otmp = ctx.enter_context(tc.tile_pool(name="otmp", bufs=3))
    for nch in range(N // P):
        op = ops.tile([P, 2, IBL], F32, tag="op", name="op")
        for h in range(2):
            nc.tensor.matmul(op[:, h, :], lhsT=comp[:, ts(nch, P)],
                             rhs=wout_sb[:, ts(h, IBL)], start=True, stop=True)
        osb = otmp.tile([P, DIM], BF16, tag="osb", name="osb")
        if nch % 2 == 0:
            nc.scalar.copy(out=osb, in_=op)
        else:
            nc.vector.tensor_copy(out=osb, in_=op)
        nc.sync.dma_start(out=outp[ts(nch, P), :], in_=osb)


def build_nc():
    nc = bacc.Bacc()
    xT = nc.declare_dram_parameter("xT", [DIM, N], BF16, isOutput=False)
    wq = nc.declare_dram_parameter("wq", [DIM, SD], BF16, isOutput=False)
    wk = nc.declare_dram_parameter("wk", [DIM, SD], BF16, isOutput=False)
    wr = nc.declare_dram_parameter("wr", [DIM, SD], BF16, isOutput=False)
    wv = nc.declare_dram_parameter("wv", [DIM, RD], BF16, isOutput=False)
    wrk = nc.declare_dram_parameter("wrk", [D, D], BF16, isOutput=False)
    wout = nc.declare_dram_parameter("wout", [SD, DIM], BF16, isOutput=False)
    outp = nc.declare_dram_parameter("outp", [N, DIM], BF16, isOutput=True)
    io = (xT[:], wq[:], wk[:], wr[:], wv[:], wrk[:], wout[:], outp[:])
    with tile.TileContext(nc) as tc:
        with ExitStack() as ctx:
            _emit(ctx, tc, io)
    nc.compile()
    return nc


_CACHE = {}


def _get_nc():
    if "nc" not in _CACHE:
        _CACHE["nc"] = build_nc()
    return _CACHE["nc"]


def make_in_maps(x, Wsq, Wsk, Wrv, Wrq, Wrk, Wout):
    x = np.asarray(x, np.float32)
    bf = ml_dtypes.bfloat16
    in_maps = []
    for c in range(NCORES):
        b = c // 4
        s0 = 2 * (c % 4)
        sl = slice(s0 * D, (s0 + 2) * D)
        in_maps.append({
            "xT": np.ascontiguousarray(x[b].T).astype(bf),
            "wq": np.ascontiguousarray(np.asarray(Wsq, np.float32)[:, sl]).astype(bf),
            "wk": np.ascontiguousarray(np.asarray(Wsk, np.float32)[:, sl]).astype(bf),
            "wr": np.ascontiguousarray(np.asarray(Wrq, np.float32)[:, sl]).astype(bf),
            "wv": np.ascontiguousarray(np.asarray(Wrv, np.float32)).astype(bf),
            "wrk": np.ascontiguousarray(np.asarray(Wrk, np.float32)).astype(bf),
            "wout": np.ascontiguousarray(np.asarray(Wout, np.float32)[sl, :]).astype(bf),
        })
    return in_maps


def combine(results):
    out = np.zeros((B, N, DIM), np.float32)
    for c in range(NCORES):
        out[c // 4] += np.asarray(results[c]["outp"], np.float32)
    return out


def kernel(x, Wsq, Wsk, Wrv, Wrq, Wrk, Wout):
    nc = _get_nc()
    in_maps = make_in_maps(x, Wsq, Wsk, Wrv, Wrq, Wrk, Wout)
    res = run_bass_kernel_spmd(nc, in_maps, list(range(NCORES))).results
    return combine(res)


def _install_ntff_shim():
    """Provide antenv.axon_hooks in images that lack it, driving NTFF
    profiling via ctypes into the injected libaxon_pjrt.so."""
    import types
    import ctypes
    import contextlib

    try:
        from antenv.axon_hooks import get_axon_ntff_profile_hook  # noqa
        return
    except ImportError:
        pass
    so_path = "/opt/axon/libaxon_pjrt.so"
    lib = ctypes.CDLL(so_path)
    if not hasattr(lib, "axon_start_nrt_profile"):
        return
    lib.axon_start_nrt_profile.argtypes = [
        ctypes.POINTER(ctypes.c_int64), ctypes.c_size_t]
    lib.axon_start_nrt_profile.restype = ctypes.c_int64
    lib.axon_stop_nrt_profile.argtypes = [ctypes.c_char_p]
    lib.axon_stop_nrt_profile.restype = ctypes.c_int64

    @contextlib.contextmanager
    def _hook(output_dir, device_ids):
        import jax
        jax.devices()
        if device_ids:
            ids = (ctypes.c_int64 * len(device_ids))(*device_ids)
            rc = lib.axon_start_nrt_profile(ids, len(device_ids))
        else:
            rc = lib.axon_start_nrt_profile(None, 0)
        if rc != 0:
            raise RuntimeError(f"axon_start_nrt_profile rc={rc}")
        try:
            yield
        finally:
            n = lib.axon_stop_nrt_profile(str(output_dir).encode())
            print(f"profile: {n} file(s) written to {output_dir}")

    import antenv
    mod = types.ModuleType("antenv.axon_hooks")
    mod.get_axon_ntff_profile_hook = lambda: _hook
    mod.set_axon_ntff_profile_hook = lambda h: None
    sys.modules["antenv.axon_hooks"] = mod
    antenv.axon_hooks = mod


def run_traced(x, Wsq, Wsk, Wrv, Wrq, Wrk, Wout, **kw):
    _install_ntff_shim()
    nc = _get_nc()
    in_maps = make_in_maps(x, Wsq, Wsk, Wrv, Wrq, Wrk, Wout)
    br = run_bass_kernel_spmd(nc, in_maps, list(range(NCORES)), trace=True, **kw)
    return combine(br.results), br


# revision 8
# speedup vs baseline: 1.0305x; 1.0305x over previous
"""Compositional attention Trainium2 Bass kernel (V5).

Sharding: 8 cores = 2 batches x 4 search-pairs.
Core c handles batch b=c//4 and searches (2*(c%4), 2*(c%4)+1); each core
produces a partial output for its 128 columns of the S*D=512 concat dim
(host sums 4 partials per batch).

V5 notes (over V4):
  - The two searches are staggered by one query block: slot t runs
    s0 on block t and s1 on block t-1. s0 finishes a slot early, so its
    whole per-query scalar chain (row reduce -> bounce -> sigmoid ->
    broadcast -> blend) overlaps s1's last attention block.
  - All four per-query row reduces of a slot (sums/diff x both
    searches) land in ONE PSUM bank on partitions 0/32/64/96 via
    distinct PE col groups, freeing banks so the chain runs inside the
    attention loop (8-bank budget: scores 4 + retrieve 2 + dW 1 + rows 1).
  - Weight DMAs are spread across the scalar/gpsimd queues so the first
    projection matmul starts right after x chunk 0 lands.
  - Output projection uses 6 PSUM banks of run-ahead.
"""

import sys

for _p in ("/opt/trn_rl_repo",):
    if _p not in sys.path:
        sys.path.insert(0, _p)

from contextlib import ExitStack

import ml_dtypes
import numpy as np

import concourse.bass as bass
import concourse.tile as tile
from concourse import bacc
from concourse import mybir
from concourse.bass import ts
from concourse.bass_utils import run_bass_kernel_spmd

B, N, DIM, S, R, D = 2, 2048, 1024, 8, 2, 64
NCORES = 8
SPC = 2          # searches per core
SD = SPC * D     # 128 (per-core slice of S*D)
RD = R * D       # 128
P = 128
IBL = 512        # query block
NIB = N // IBL   # 4
KC = DIM // P    # 8
NJT = N // P     # 16 key tiles
F32 = mybir.dt.float32
BF16 = mybir.dt.bfloat16
SCALE = float(D) ** -0.5
AF = mybir.ActivationFunctionType
ALU = mybir.AluOpType


def _emit(ctx: ExitStack, tc: tile.TileContext, io):
    nc = tc.nc
    xT, wq, wk, wr, wv, wrk, wout, outp = io

    singles = ctx.enter_context(tc.tile_pool(name="singles", bufs=1))
    ones_b = singles.tile([P, 1], BF16)
    nc.vector.memset(ones_b, 1.0)

    wq_sb = singles.tile([P, KC, SD], BF16)
    wk_sb = singles.tile([P, KC, SD], BF16)
    wr_sb = singles.tile([P, KC, SD], BF16)
    wv_sb = singles.tile([P, KC, RD], BF16)
    wrk2 = singles.tile([P, D], BF16)   # Wrk twice: rows 0:64 and 64:128
    wout_sb = singles.tile([P, DIM], BF16)

    acts = ctx.enter_context(tc.tile_pool(name="acts", bufs=1))
    qT = acts.tile([P, N], BF16)
    kT = acts.tile([P, N], BF16)
    rqT = acts.tile([P, N], BF16)
    vnat = acts.tile([P, NJT, RD], BF16)   # [key-part, key-tile, r*d]
    ret0 = acts.tile([P, N], BF16)         # search0 retrievedT (unnormalized)
    ret1 = acts.tile([P, N], BF16)         # search1
    rsh = acts.tile([P, N], BF16)          # [0:64]=s0 r1 shifted dn, [64:128]=s1 r0 up
    dT = acts.tile([P, N], BF16)           # r0-r1 per search (s0 rows 0:64)
    dprod = acts.tile([P, N], BF16)        # rq * (d @ Wrk)
    red0 = acts.tile([P, N], BF16)         # per-key-part exp sums
    red1 = acts.tile([P, N], BF16)
    bc0 = acts.tile([P, N], BF16)          # broadcast sig*inv
    bc1 = acts.tile([P, N], BF16)          # broadcast inv
    comp = acts.tile([P, N], BF16)
    rets = (ret0, ret1)
    reds = (red0, red1)

    # ---------------- projections ----------------
    with tc.tile_pool(name="xpool", bufs=1) as xpool, \
         tc.tile_pool(name="pja", bufs=1, space="PSUM") as pja, \
         tc.tile_pool(name="pjb", bufs=2, space="PSUM") as pjb:
        xs = xpool.tile([P, KC, N], BF16)
        xr = xT.rearrange("(kc p) n -> p kc n", p=P)
        # x chunk 0 + pass-A weights first, each on its own queue
        nc.sync.dma_start(out=xs[:, 0, :], in_=xr[:, 0, :])
        nc.scalar.dma_start(out=wk_sb,
                            in_=wk.rearrange("(kc p) m -> p kc m", p=P))
        nc.gpsimd.dma_start(out=wv_sb,
                            in_=wv.rearrange("(kc p) m -> p kc m", p=P))
        for k in range(1, KC):
            nc.sync.dma_start(out=xs[:, k, :], in_=xr[:, k, :])
        nc.scalar.dma_start(out=wq_sb,
                            in_=wq.rearrange("(kc p) m -> p kc m", p=P))
        nc.gpsimd.dma_start(out=wr_sb,
                            in_=wr.rearrange("(kc p) m -> p kc m", p=P))
        nc.scalar.dma_start(out=wrk2[0:64, :], in_=wrk)
        nc.scalar.dma_start(out=wrk2[64:128, :], in_=wrk)
        nc.gpsimd.dma_start(out=wout_sb, in_=wout)

        vtmp = xpool.tile([P, N], BF16)
        # pass A: kT + vT (8 banks), k-ordered so MM k waits only chunk k
        kps = [pja.tile([P, IBL], F32, tag="pk", name=f"pk{ib}")
               for ib in range(NIB)]
        vps = [pja.tile([P, IBL], F32, tag="pv", name=f"pv{ib}")
               for ib in range(NIB)]
        for k in range(KC):
            for ib in range(NIB):
                nc.tensor.matmul(kps[ib], lhsT=wk_sb[:, k, :],
                                 rhs=xs[:, k, ts(ib, IBL)],
                                 start=(k == 0), stop=(k == KC - 1))
                nc.tensor.matmul(vps[ib], lhsT=wv_sb[:, k, :],
                                 rhs=xs[:, k, ts(ib, IBL)],
                                 start=(k == 0), stop=(k == KC - 1))
        for ib in range(NIB):
            nc.vector.tensor_copy(out=kT[:, ts(ib, IBL)], in_=kps[ib])
            nc.scalar.copy(out=vtmp[:, ts(ib, IBL)], in_=vps[ib])
            for h in range(IBL // P):
                jt = ib * (IBL // P) + h
                nc.scalar.dma_start_transpose(vnat[:, jt, :], vtmp[:, ts(jt, P)])
        # pass B: qT + rqT, ib-ordered so attention slot 0 can start early
        for ib in range(NIB):
            qp = pjb.tile([P, IBL], F32, tag="pq", name="pq")
            rp = pjb.tile([P, IBL], F32, tag="pr", name="pr")
            for k in range(KC):
                nc.tensor.matmul(qp, lhsT=wq_sb[:, k, :],
                                 rhs=xs[:, k, ts(ib, IBL)],
                                 start=(k == 0), stop=(k == KC - 1))
                nc.tensor.matmul(rp, lhsT=wr_sb[:, k, :],
                                 rhs=xs[:, k, ts(ib, IBL)],
                                 start=(k == 0), stop=(k == KC - 1))
            nc.vector.tensor_copy(out=qT[:, ts(ib, IBL)], in_=qp)
            nc.scalar.copy(out=rqT[:, ts(ib, IBL)], in_=rp)

    # DRAM bounce buffers for per-query scalars
    dramp = ctx.enter_context(tc.tile_pool(name="dramp", bufs=1, space="DRAM"))
    sums_dr = [dramp.tile([N], F32, tag=f"sums{si}", name=f"sums{si}")
               for si in range(SPC)]
    diff_dr = [dramp.tile([N], F32, tag=f"diff{si}", name=f"diff{si}")
               for si in range(SPC)]
    a0_dr = [dramp.tile([N], BF16, tag=f"a0{si}", name=f"a0d{si}")
             for si in range(SPC)]
    a1_dr = [dramp.tile([N], BF16, tag=f"a1{si}", name=f"a1d{si}")
             for si in range(SPC)]

    etmp = ctx.enter_context(tc.tile_pool(name="etmp", bufs=2))
    s128 = [etmp.tile([P, N // P], F32, tag=f"s128_{si}", name=f"s128_{si}")
            for si in range(SPC)]
    d128 = [etmp.tile([P, N // P], F32, tag=f"d128_{si}", name=f"d128_{si}")
            for si in range(SPC)]
    inv = [etmp.tile([P, N // P], F32, tag=f"inv{si}", name=f"inv{si}")
           for si in range(SPC)]
    t16 = [etmp.tile([P, N // P], F32, tag=f"t16_{si}", name=f"t16_{si}")
           for si in range(SPC)]
    ra0 = [etmp.tile([P, N // P], F32, tag=f"ra0_{si}", name=f"ra0_{si}")
           for si in range(SPC)]
    a0b = [etmp.tile([P, N // P], BF16, tag=f"a0b{si}", name=f"a0b{si}")
           for si in range(SPC)]
    a1b = [etmp.tile([P, N // P], BF16, tag=f"a1b{si}", name=f"a1b{si}")
           for si in range(SPC)]
    t1 = etmp.tile([P, N], BF16, tag="t1")
    t2 = etmp.tile([P, N], BF16, tag="t2")

    def chain(si, dq):
        """Per-query scalar chain for one search (through a0/a1 DRAM rows).
        dq is the DMA queue to ride."""
        dq.dma_start(out=s128[si],
                     in_=sums_dr[si].rearrange("(p f) -> p f", p=P))
        dq.dma_start(out=d128[si],
                     in_=diff_dr[si].rearrange("(p f) -> p f", p=P))
        nc.vector.reciprocal(inv[si], s128[si])
        nc.vector.tensor_tensor(t16[si], d128[si], inv[si], ALU.mult)
        nc.scalar.activation(out=ra0[si], in_=t16[si], func=AF.Sigmoid,
                             scale=SCALE)
        nc.vector.tensor_tensor(a0b[si], ra0[si], inv[si], ALU.mult)
        nc.vector.tensor_copy(out=a1b[si], in_=inv[si])
        dq.dma_start(out=a0_dr[si].rearrange("(p f) -> p f", p=P),
                     in_=a0b[si])
        dq.dma_start(out=a1_dr[si].rearrange("(p f) -> p f", p=P),
                     in_=a1b[si])

    # ---------------- attention ----------------
    # PSUM: scores 4 + ret 2 + dW 1 + rows 1 = 8 banks.
    with tc.tile_pool(name="expp", bufs=2) as expp, \
         tc.tile_pool(name="trp1", bufs=2) as trp1, \
         tc.tile_pool(name="trp2", bufs=2) as trp2, \
         tc.tile_pool(name="trp3", bufs=2) as trp3, \
         tc.tile_pool(name="scp", bufs=2, space="PSUM") as scp, \
         tc.tile_pool(name="retp", bufs=1, space="PSUM") as retp, \
         tc.tile_pool(name="dwp", bufs=1, space="PSUM") as dwp, \
         tc.tile_pool(name="rowp", bufs=1, space="PSUM") as rowp:
        for ib in range(NIB):
            act = [(0, ib), (1, ib)]
            ets = expp.tile([P, NJT, SPC, IBL], BF16, tag="exp", name="exp")
            rt = {si: retp.tile([P, IBL], F32, tag=f"rt{si}", name=f"rt{si}")
                  for si, _ in act}
            for jt in range(NJT):
                sp = scp.tile([P, SPC, IBL], F32, tag="sc", name="sc")
                for si, _ in act:
                    lo = 64 * si
                    nc.tensor.matmul(
                        sp[:, si, :],
                        lhsT=kT[lo:lo + 64, ts(jt, P)],
                        rhs=qT[lo:lo + 64, ts(ib, IBL)],
                        start=True, stop=True,
                    )
                nc.scalar.activation(out=ets[:, jt, :, :], in_=sp,
                                     func=AF.Exp, scale=SCALE)
                for si, _ in act:
                    nc.tensor.matmul(
                        rt[si], lhsT=vnat[:, jt, :], rhs=ets[:, jt, si, :],
                        start=(jt == 0), stop=(jt == NJT - 1),
                        skip_group_check=True,
                    )
            dwps = dwp.tile([P, IBL], F32, tag="dw", name="dw")
            rowsps = rowp.tile([P, IBL], F32, tag="rows", name="rowsps")
            rowsb = etmp.tile([P, IBL], F32, tag="rowsb", name="rowsb")
            for si, ib in act:
                lo = 64 * si
                ibs = ts(ib, IBL)
                # retrieved evac + partition shift + d = r0 - r1
                if si == 0:
                    nc.vector.tensor_copy(out=ret0[:, ibs], in_=rt[0])
                    nc.gpsimd.dma_start(out=rsh[0:64, ibs],
                                        in_=ret0[64:128, ibs])
                    nc.vector.tensor_tensor(dT[0:64, ibs], ret0[0:64, ibs],
                                            rsh[0:64, ibs], ALU.subtract)
                else:
                    nc.scalar.copy(out=ret1[:, ibs], in_=rt[1])
                    nc.gpsimd.dma_start(out=rsh[64:128, ibs],
                                        in_=ret1[0:64, ibs])
                    nc.vector.tensor_tensor(dT[64:128, ibs],
                                            rsh[64:128, ibs],
                                            ret1[64:128, ibs], ALU.subtract)
                # dW = Wrk^T @ d (diagonal tiles), dprod = rq * dW
                nc.tensor.matmul(dwps[lo:lo + 64, :], lhsT=wrk2[lo:lo + 64, :],
                                 rhs=dT[lo:lo + 64, ibs], start=True, stop=True)
                nc.vector.tensor_tensor(dprod[lo:lo + 64, ibs],
                                        rqT[lo:lo + 64, ibs],
                                        dwps[lo:lo + 64, :], ALU.mult)
                # denominator tree: lvl1+2 DVE, lvl3+4 GpSimd
                g1 = trp1.tile([P, NJT // 2, IBL], BF16, tag="g1",
                               name=f"g1_{si}")
                nc.vector.tensor_tensor(g1, ets[:, 0:8, si, :],
                                        ets[:, 8:16, si, :], ALU.add)
                g2 = trp2.tile([P, NJT // 4, IBL], BF16, tag="g2",
                               name=f"g2_{si}")
                nc.vector.tensor_tensor(g2, g1[:, 0:4, :], g1[:, 4:8, :],
                                        ALU.add)
                g3 = trp3.tile([P, NJT // 8, IBL], BF16, tag="g3",
                               name=f"g3_{si}")
                nc.gpsimd.tensor_tensor(g3, g2[:, 0:2, :], g2[:, 2:4, :],
                                        ALU.add)
                nc.gpsimd.tensor_tensor(reds[si][:, ibs],
                                        g3[:, 0, :], g3[:, 1, :], ALU.add)
                # per-query rows into one bank: sums@(col 64*si),
                # diff@(col 64*si+32)
                nc.tensor.matmul(rowsps[lo:lo + 1, :], lhsT=ones_b,
                                 rhs=reds[si][:, ibs], start=True, stop=True,
                                 tile_position=(0, lo))
                nc.tensor.matmul(rowsps[lo + 32:lo + 33, :],
                                 lhsT=ones_b[lo:lo + 64, :],
                                 rhs=dprod[lo:lo + 64, ibs], start=True,
                                 stop=True, tile_position=(lo, lo + 32))
                if si == 0:
                    nc.vector.tensor_copy(out=rowsb[0:1, :],
                                          in_=rowsps[0:1, :])
                    nc.vector.tensor_copy(out=rowsb[32:33, :],
                                          in_=rowsps[32:33, :])
                    nc.sync.dma_start(out=sums_dr[0][None, ibs],
                                      in_=rowsb[0:1, :])
                    nc.sync.dma_start(out=diff_dr[0][None, ibs],
                                      in_=rowsb[32:33, :])
                else:
                    nc.scalar.copy(out=rowsb[64:65, :], in_=rowsps[64:65, :])
                    nc.scalar.copy(out=rowsb[96:97, :], in_=rowsps[96:97, :])
                    nc.scalar.dma_start(out=sums_dr[1][None, ibs],
                                        in_=rowsb[64:65, :])
                    nc.scalar.dma_start(out=diff_dr[1][None, ibs],
                                        in_=rowsb[96:97, :])

    # ---------------- epilogue: chains + blends ----------------
    chain(0, nc.sync)
    chain(1, nc.scalar)
    # broadcasts split into N/2 halves across 3 queues, blends per half on
    # DVE so compute overlaps the broadcast transfers
    H = N // 2
    bq = {(0, 0): nc.sync, (0, 1): nc.scalar, (1, 0): nc.gpsimd,
          (1, 1): nc.sync}
    for hh in range(2):
        hs = ts(hh, H)
        for si in range(SPC):
            lo = 64 * si
            bq[(si, 0)].dma_start(
                out=bc0[lo:lo + 64, hs],
                in_=a0_dr[si][None, hs].to_broadcast([64, H]))
            bq[(si, 1)].dma_start(
                out=bc1[lo:lo + 64, hs],
                in_=a1_dr[si][None, hs].to_broadcast([64, H]))
        # comp = inv*r1 + (sig*inv)*(r0-r1)
        for si in range(SPC):
            lo = 64 * si
            r1ap = rsh[0:64, hs] if si == 0 else ret1[64:128, hs]
            nc.vector.tensor_tensor(t1[lo:lo + 64, hs], bc0[lo:lo + 64, hs],
                                    dT[lo:lo + 64, hs], ALU.mult)
            nc.vector.tensor_tensor(t2[lo:lo + 64, hs], bc1[lo:lo + 64, hs],
                                    r1ap, ALU.mult)
            nc.vector.tensor_tensor(comp[lo:lo + 64, hs], t1[lo:lo + 64, hs],
                                    t2[lo:lo + 64, hs], ALU.add)

    # ---------------- output projection ----------------
    ops = ctx.enter_context(tc.tile_pool(name="ops", bufs=3, space="PSUM"))
    otmp = ctx.enter_context(tc.tile_pool(name="otmp", bufs=3))
    for nch in range(N // P):
        op = ops.tile([P, 2, IBL], F32, tag="op", name="op")
        for h in range(2):
            nc.tensor.matmul(op[:, h, :], lhsT=comp[:, ts(nch, P)],
                             rhs=wout_sb[:, ts(h, IBL)], start=True, stop=True)
        osb = otmp.tile([P, DIM], BF16, tag="osb", name="osb")
        if nch % 2 == 0:
            nc.scalar.copy(out=osb, in_=op)
        else:
            nc.vector.tensor_copy(out=osb, in_=op)
        (nc.sync if nch % 2 == 0 else nc.scalar).dma_start(
            out=outp[ts(nch, P), :], in_=osb)


def build_nc():
    nc = bacc.Bacc()
    xT = nc.declare_dram_parameter("xT", [DIM, N], BF16, isOutput=False)
    wq = nc.declare_dram_parameter("wq", [DIM, SD], BF16, isOutput=False)
    wk = nc.declare_dram_parameter("wk", [DIM, SD], BF16, isOutput=False)
    wr = nc.declare_dram_parameter("wr", [DIM, SD], BF16, isOutput=False)
    wv = nc.declare_dram_parameter("wv", [DIM, RD], BF16, isOutput=False)
    wrk = nc.declare_dram_parameter("wrk", [D, D], BF16, isOutput=False)
    wout = nc.declare_dram_parameter("wout", [SD, DIM], BF16, isOutput=False)
    outp = nc.declare_dram_parameter("outp", [N, DIM], BF16, isOutput=True)
    io = (xT[:], wq[:], wk[:], wr[:], wv[:], wrk[:], wout[:], outp[:])
    with tile.TileContext(nc) as tc:
        with ExitStack() as ctx:
            _emit(ctx, tc, io)
    nc.compile()
    return nc


_CACHE = {}


def _get_nc():
    if "nc" not in _CACHE:
        _CACHE["nc"] = build_nc()
    return _CACHE["nc"]


def make_in_maps(x, Wsq, Wsk, Wrv, Wrq, Wrk, Wout):
    x = np.asarray(x, np.float32)
    bf = ml_dtypes.bfloat16
    in_maps = []
    for c in range(NCORES):
        b = c // 4
        s0 = 2 * (c % 4)
        sl = slice(s0 * D, (s0 + 2) * D)
        in_maps.append({
            "xT": np.ascontiguousarray(x[b].T).astype(bf),
            "wq": np.ascontiguousarray(np.asarray(Wsq, np.float32)[:, sl]).astype(bf),
            "wk": np.ascontiguousarray(np.asarray(Wsk, np.float32)[:, sl]).astype(bf),
            "wr": np.ascontiguousarray(np.asarray(Wrq, np.float32)[:, sl]).astype(bf),
            "wv": np.ascontiguousarray(np.asarray(Wrv, np.float32)).astype(bf),
            "wrk": np.ascontiguousarray(np.asarray(Wrk, np.float32)).astype(bf),
            "wout": np.ascontiguousarray(np.asarray(Wout, np.float32)[sl, :]).astype(bf),
        })
    return in_maps


def combine(results):
    out = np.zeros((B, N, DIM), np.float32)
    for c in range(NCORES):
        out[c // 4] += np.asarray(results[c]["outp"], np.float32)
    return out


def kernel(x, Wsq, Wsk, Wrv, Wrq, Wrk, Wout):
    nc = _get_nc()
    in_maps = make_in_maps(x, Wsq, Wsk, Wrv, Wrq, Wrk, Wout)
    res = run_bass_kernel_spmd(nc, in_maps, list(range(NCORES))).results
    return combine(res)


def _install_ntff_shim():
    """Provide antenv.axon_hooks in images that lack it, driving NTFF
    profiling via ctypes into the injected libaxon_pjrt.so."""
    import types
    import ctypes
    import contextlib

    try:
        from antenv.axon_hooks import get_axon_ntff_profile_hook  # noqa
        return
    except ImportError:
        pass
    so_path = "/opt/axon/libaxon_pjrt.so"
    lib = ctypes.CDLL(so_path)
    if not hasattr(lib, "axon_start_nrt_profile"):
        return
    lib.axon_start_nrt_profile.argtypes = [
        ctypes.POINTER(ctypes.c_int64), ctypes.c_size_t]
    lib.axon_start_nrt_profile.restype = ctypes.c_int64
    lib.axon_stop_nrt_profile.argtypes = [ctypes.c_char_p]
    lib.axon_stop_nrt_profile.restype = ctypes.c_int64

    @contextlib.contextmanager
    def _hook(output_dir, device_ids):
        import jax
        jax.devices()
        if device_ids:
            ids = (ctypes.c_int64 * len(device_ids))(*device_ids)
            rc = lib.axon_start_nrt_profile(ids, len(device_ids))
        else:
            rc = lib.axon_start_nrt_profile(None, 0)
        if rc != 0:
            raise RuntimeError(f"axon_start_nrt_profile rc={rc}")
        try:
            yield
        finally:
            n = lib.axon_stop_nrt_profile(str(output_dir).encode())
            print(f"profile: {n} file(s) written to {output_dir}")

    import antenv
    mod = types.ModuleType("antenv.axon_hooks")
    mod.get_axon_ntff_profile_hook = lambda: _hook
    mod.set_axon_ntff_profile_hook = lambda h: None
    sys.modules["antenv.axon_hooks"] = mod
    antenv.axon_hooks = mod


def run_traced(x, Wsq, Wsk, Wrv, Wrq, Wrk, Wout, **kw):
    _install_ntff_shim()
    nc = _get_nc()
    in_maps = make_in_maps(x, Wsq, Wsk, Wrv, Wrq, Wrk, Wout)
    br = run_bass_kernel_spmd(nc, in_maps, list(range(NCORES)), trace=True, **kw)
    return combine(br.results), br
